# revision 13
# baseline (speedup 1.0000x reference)
"""Trainium2 Bass kernel for virtual-node GAT attention (gnn_message_passing).

Reference semantics (N=100000, C=64, D=512, F=256):
    gh  = graph_node @ W            # (N, F)
    vh  = virtual_node @ W          # (C, F)
    e   = gh @ a1 + (vh @ a2)^T     # (N, C)
    e   = leaky_relu(e, 0.2)
    att = softmax(e, axis=1)
    out = att @ vh                  # (N, F)

Algebraic identity: gh only enters via gh @ a1 = graph_node @ (W @ a1), so
the (N,D)@(D,F) matmul is never needed. Host precomputes the tiny shared
tables w1 = W@a1 (D,), vh (C,F), t = vh@a2 (C,).

Transposed device pipeline: the host stages x TRANSPOSED (xT [D, rows],
fp16), so every per-row stage runs with rows on the matmul free dim and no
on-chip transpose is ever needed:
  sT   = w1rep^T @ xT          PE: 4 accumulating 128-contraction matmuls
                               per 512 rows, lhsT = w1 chunk replicated 64
                               wide -> sT in PSUM [64 (redundant), rows]
  eT   = Prelu(sT + t)         ACT: one op per 1024 rows; t is a per-
                               partition bias [64,1] in this layout
  pexpT= Exp(eT - 10.5)        ACT: shift keeps exp(e) inside fp16 range
  h|z  = pexpT^T @ [vh | 1]    PE: pexpT slices [64,128] are ALREADY in
                               lhsT layout; ones-column gives z for free
  osb  = copy h|z              DVE: strided PSUM->SBUF fp16 casts
Host divides h by z (softmax denominator) and casts to fp32; the shift
cancels in the division. fp16 end-to-end rel err ~4e-3 (gate is 2e-2).

Output rows are stored PARTITION-MAJOR in HBM (hbm row p*NCHUNK+q holds
graph row q*128+p) so each partition writes one contiguous ~8KB run per
group instead of 100 separate 514B packets; the host un-permutes with one
cheap reshape. Everything streams fp16: 19.7 MB HBM traffic per core
(13.1 in + 6.6 out) vs 39 MB for the fp32 baseline.

Sharding: graph_node rows split evenly across 8 cores (data parallel),
small tables replicated. No cross-device communication.
"""

import numpy as np

N, D, F, C = 100000, 512, 256, 64
NCORES = 8
SHARD = N // NCORES            # 12500 rows per core
P = 128                        # partitions
RPP = 512                      # rows per pair (one psS bank of fp32)
NPAIR = (SHARD + RPP - 1) // RPP   # 25
PADR = NPAIR * RPP             # 12800 rows per core (zero-padded)
NCHUNK = PADR // P             # 100 output chunks of 128 rows
FA = F + 1                     # 257: h columns + z (softmax denom)
# group sizes in pairs: small first group -> compute starts sooner; small
# tail -> short drain after the final load. Even sizes so pairs batch into
# 2-pair blocks (one Prelu/Exp per 1024 rows); the final pair runs alone.
GROUPS = [1, 1, 2, 4, 4, 4, 4, 2, 2, 1]
assert sum(GROUPS) == NPAIR
ALPHA = 0.2
MSHIFT = -10.5                 # exp argument shift (cancels in softmax);
                               # keeps h = pexp@vh under fp16 max (~9e3
                               # worst row) and z above fp16 normal min

_CACHE = {}


def _build_nc():
    import concourse.bacc as bacc
    import concourse.mybir as mybir
    import concourse.tile as tile

    fp32 = mybir.dt.float32
    fp16 = mybir.dt.float16
    Act = mybir.ActivationFunctionType

    nc = bacc.Bacc("TRN2", target_bir_lowering=False, debug=False,
                   num_devices=NCORES)
    xT = nc.dram_tensor("xT", [D, PADR], fp16, kind="ExternalInput").ap()
    w1rep = nc.dram_tensor("w1rep", [D, C], fp16, kind="ExternalInput").ap()
    tbias = nc.dram_tensor("tbias", [2 * C, 2], fp32, kind="ExternalInput").ap()
    vha = nc.dram_tensor("vha", [2, 2 * C, FA], fp16, kind="ExternalInput").ap()
    out = nc.dram_tensor("out", [PADR, FA], fp16, kind="ExternalOutput").ap()

    # device-side views:
    #   xT as [p=128, chunk=4, rows]  (partition p owns d = c*128 + p)
    xTv = xT.rearrange("(c p) r -> p c r", c=4, p=P)
    #   out partition-major: hbm row p*NCHUNK + q <-> graph row q*128 + p
    outv = out.rearrange("(p q) f -> p q f", q=NCHUNK)

    with tile.TileContext(nc) as tc:
        with (
            tc.tile_pool(name="const", bufs=1) as constp,
            tc.tile_pool(name="xin", bufs=3) as xp,
            tc.tile_pool(name="evec", bufs=2) as ep,
            tc.tile_pool(name="pexp", bufs=2) as pp,
            tc.tile_pool(name="osb", bufs=3) as op_,
            tc.tile_pool(name="psS", bufs=2, space="PSUM") as psS,
            tc.tile_pool(name="psH", bufs=2, space="PSUM") as psH,
        ):
            w1_sb = constp.tile([P, 4, C], fp16)
            nc.sync.dma_start(out=w1_sb,
                              in_=w1rep.rearrange("(c p) f -> p c f", c=4))
            t_sb = constp.tile([2 * C, 2], fp32)
            nc.sync.dma_start(out=t_sb, in_=tbias)
            vh_sb = constp.tile([2 * C, 2, FA], fp16)
            nc.sync.dma_start(out=vh_sb,
                              in_=vha.rearrange("v p f -> p v f"))

            # one block per group (<=4 pairs). Pairs stack two-deep on
            # PSUM partitions: pair bb lives at partitions 64*(bb//2).. and
            # bank slot bb%2, so a 4-pair block fills [128, 2, 512] = 2
            # banks and ONE Prelu/Exp covers all 2048 rows. The s-stage of
            # group g+1 is emitted before the softmax/h-stage of group g so
            # the PE never stalls on ACT.
            nG = len(GROUPS)
            gbase = [sum(GROUPS[:g]) for g in range(nG)]
            state = {}

            def emit_load(g):
                # one transfer per group, issued a full group ahead of use
                r0, r1 = gbase[g] * RPP, (gbase[g] + GROUPS[g]) * RPP
                xt = xp.tile([P, 4, GROUPS[g] * RPP], fp16, tag="xt")
                nc.sync.dma_start(out=xt, in_=xTv[:, :, r0:r1])
                state[g] = xt

            def emit_s(g):
                nb = GROUPS[g]
                xt = state[g]
                # sT[p, r] = x[r, :] . w1 (64 partition copies feed Prelu's
                # bias layout); chunk-major so consecutive matmuls reuse
                # the same loaded w1 chunk (same-weight matmuls skip the
                # weight-load cost)
                psum_s = psS.tile([P, 2, RPP], fp32)
                for c in range(4):
                    for bb in range(nb):
                        hb, sb = 64 * (bb // 2), bb % 2
                        nc.tensor.matmul(psum_s[hb:hb + C, sb, :],
                                         w1_sb[:, c, :],
                                         xt[:, c, bb * RPP:(bb + 1) * RPP],
                                         start=(c == 0), stop=(c == 3))
                state[g] = (xt, psum_s)

            def emit_rest(g):
                nb = GROUPS[g]
                _, psum_s = state.pop(g)
                nparts = C if nb <= 2 else P
                nslots = min(nb, 2)
                osb = op_.tile([P, nb * (RPP // P), FA], fp16, tag="osb",
                               name="osb")
                # eT = leaky_relu(sT + t_j): t is a per-partition bias
                eT = ep.tile([P, 2, RPP], fp16, tag="eT")
                nc.scalar.activation(out=eT[:nparts, :nslots, :],
                                     in_=psum_s[:nparts, :nslots, :],
                                     func=Act.Prelu,
                                     bias=t_sb[:nparts, 0:1], scale=1.0,
                                     alpha=ALPHA)
                # pexpT = exp(eT + MSHIFT), shifted into fp16-safe range;
                # the shift cancels in h/z on host
                pexpT = pp.tile([P, 2, RPP], fp16, tag="pexpT")
                nc.scalar.activation(out=pexpT[:nparts, :nslots, :],
                                     in_=eT[:nparts, :nslots, :],
                                     func=Act.Exp, bias=t_sb[:nparts, 1:2],
                                     scale=1.0)
                # h|z chunks of 128 rows: lhsT = pexpT slice (already
                # transposed layout), rhs = [vh | ones]
                for half in range(2 * nb):
                    bb = half // 2
                    hv, sb = bb // 2, bb % 2
                    ps_h = psH.tile([P, 2, RPP], fp32, name="ps_h",
                                    tag="psH")
                    for kk in range(2):
                        q = (half % 2) * 2 + kk
                        # full-height lhsT (both stacked pair-groups); the
                        # rhs variant hv zero-masks the other group's rows
                        nc.tensor.matmul(
                            ps_h[:, kk, :FA],
                            pexpT[:, sb, q * P:(q + 1) * P] if nb > 2
                            else pexpT[:C, sb, q * P:(q + 1) * P],
                            vh_sb[:, hv, :] if nb > 2
                            else vh_sb[:C, 0, :], start=True, stop=True)
                    oq = bb * 4 + (half % 2) * 2
                    nc.vector.tensor_copy(osb[:, oq:oq + 2, :],
                                          ps_h[:, :, :FA])
                q0 = gbase[g] * (RPP // P)
                nc.scalar.dma_start(out=outv[:, q0:q0 + 4 * nb, :], in_=osb)

            emit_load(0)
            emit_load(1)
            emit_s(0)
            for g in range(1, nG + 1):
                if g < nG:
                    if g + 1 < nG:
                        emit_load(g + 1)
                    emit_s(g)
                emit_rest(g - 1)

    nc.compile()
    return nc


def _get_nc():
    if "nc" not in _CACHE:
        _CACHE["nc"] = _build_nc()
    return _CACHE["nc"]


def _prep_inputs(graph_node, virtual_node, W, a):
    f32, f16 = np.float32, np.float16
    W = np.asarray(W, f32)
    a = np.asarray(a, f32)
    a1 = a[:F, 0]
    a2 = a[F:, 0]
    w1 = (W @ a1).astype(f32)                             # (D,)
    vh = (np.asarray(virtual_node, f32) @ W).astype(f32)  # (C, F)
    t = (vh @ a2).astype(f32)                             # (C,)
    w1rep = np.ascontiguousarray(
        np.broadcast_to(w1[:, None].astype(f16), (D, C)))
    t2 = np.concatenate([t, t])
    tbias = np.stack([t2, np.full((2 * C,), MSHIFT, f32)], axis=1)
    tbias = np.ascontiguousarray(tbias, dtype=f32)
    vh1 = np.concatenate([vh, np.ones((C, 1), f32)], axis=1).astype(f16)
    z64 = np.zeros_like(vh1)
    vha = np.ascontiguousarray(np.stack([
        np.concatenate([vh1, z64], axis=0),      # top pair-group live
        np.concatenate([z64, vh1], axis=0),      # bottom pair-group live
    ]))

    X = np.asarray(graph_node, f32).astype(f16)
    in_maps = []
    for c in range(NCORES):
        xT = np.zeros((D, PADR), f16)
        xT[:, :SHARD] = X[c * SHARD:(c + 1) * SHARD].T
        in_maps.append({"xT": xT, "w1rep": w1rep, "tbias": tbias,
                        "vha": vha})
    return in_maps


def _postprocess(res):
    outs = []
    for c in range(NCORES):
        o = res[c]["out"]                                 # (PADR, FA) fp16
        # un-permute partition-major rows: hbm row p*NCHUNK+q -> q*128+p
        o = np.ascontiguousarray(
            o.reshape(P, NCHUNK, FA).transpose(1, 0, 2).reshape(PADR, FA)
        )[:SHARD].astype(np.float32)
        outs.append(o[:, :F] / o[:, F:F + 1])
    return np.concatenate(outs, axis=0)


def _run(inputs, trace=False, **trace_kwargs):
    from concourse.bass_utils import run_bass_kernel_spmd

    nc = _get_nc()
    in_maps = _prep_inputs(**inputs)
    res = run_bass_kernel_spmd(nc, in_maps, list(range(NCORES)),
                               trace=trace, **trace_kwargs)
    out = _postprocess([res.results[c] for c in range(NCORES)])
    return out, res


def kernel(**inputs) -> np.ndarray:
    out, _ = _run(inputs)
    return out


# revision 14
# speedup vs baseline: 1.0544x; 1.0544x over previous
"""Trainium2 Bass kernel for virtual-node GAT attention (gnn_message_passing).

Reference semantics (N=100000, C=64, D=512, F=256):
    gh  = graph_node @ W            # (N, F)
    vh  = virtual_node @ W          # (C, F)
    e   = gh @ a1 + (vh @ a2)^T     # (N, C)
    e   = leaky_relu(e, 0.2)
    att = softmax(e, axis=1)
    out = att @ vh                  # (N, F)

Algebraic identity: gh only enters via gh @ a1 = graph_node @ (W @ a1), so
the (N,D)@(D,F) matmul is never needed. Host precomputes the tiny shared
tables w1 = W@a1 (D,), vh (C,F), t = vh@a2 (C,).

Transposed device pipeline: the host stages x TRANSPOSED (xT [D, rows],
fp16), so every per-row stage runs with rows on the matmul free dim and no
on-chip transpose is ever needed:
  sT   = w1rep^T @ xT          PE: 4 accumulating 128-contraction matmuls
                               per 512 rows, lhsT = w1 chunk replicated 64
                               wide -> sT in PSUM [64 (redundant), rows]
  eT   = Prelu(sT + t)         ACT: one op per 1024 rows; t is a per-
                               partition bias [64,1] in this layout
  pexpT= Exp(eT - 10.5)        ACT: shift keeps exp(e) inside fp16 range
  h|z  = pexpT^T @ [vh | 1]    PE: pexpT slices [64,128] are ALREADY in
                               lhsT layout; ones-column gives z for free
  osb  = copy h|z              DVE: strided PSUM->SBUF fp16 casts
Host divides h by z (softmax denominator) and casts to fp32; the shift
cancels in the division. fp16 end-to-end rel err ~4e-3 (gate is 2e-2).

Output rows are stored PARTITION-MAJOR in HBM (hbm row p*NCHUNK+q holds
graph row q*128+p) so each partition writes one contiguous ~8KB run per
group instead of 100 separate 514B packets; the host un-permutes with one
cheap reshape. Everything streams fp16: 19.7 MB HBM traffic per core
(13.1 in + 6.6 out) vs 39 MB for the fp32 baseline.

Sharding: graph_node rows split evenly across 8 cores (data parallel),
small tables replicated. No cross-device communication.
"""

import numpy as np

N, D, F, C = 100000, 512, 256, 64
NCORES = 8
SHARD = N // NCORES            # 12500 rows per core
P = 128                        # partitions
RPP = 512                      # rows per pair (one psS bank of fp32)
NPAIR = (SHARD + RPP - 1) // RPP   # 25
PADR = NPAIR * RPP             # 12800 rows per core (zero-padded)
NCHUNK = PADR // P             # 100 output chunks of 128 rows
FA = F + 1                     # 257: h columns + z (softmax denom)
# group sizes in pairs: small first group -> compute starts sooner; small
# tail -> short drain after the final load. Even sizes so pairs batch into
# 2-pair blocks (one Prelu/Exp per 1024 rows); the final pair runs alone.
GROUPS = [2, 4, 4, 4, 4, 4, 2, 1]
assert sum(GROUPS) == NPAIR
ALPHA = 0.2
MSHIFT = -10.5                 # exp argument shift (cancels in softmax);
                               # keeps h = pexp@vh under fp16 max (~9e3
                               # worst row) and z above fp16 normal min

_CACHE = {}


def _build_nc():
    import concourse.bacc as bacc
    import concourse.mybir as mybir
    import concourse.tile as tile

    fp32 = mybir.dt.float32
    fp16 = mybir.dt.float16
    Act = mybir.ActivationFunctionType

    nc = bacc.Bacc("TRN2", target_bir_lowering=False, debug=False,
                   num_devices=NCORES)
    xT = nc.dram_tensor("xT", [D, PADR], fp16, kind="ExternalInput").ap()
    w1rep = nc.dram_tensor("w1rep", [D, C], fp16, kind="ExternalInput").ap()
    tbias = nc.dram_tensor("tbias", [2 * C, 2], fp32, kind="ExternalInput").ap()
    vha = nc.dram_tensor("vha", [2, 2 * C, FA], fp16, kind="ExternalInput").ap()
    out = nc.dram_tensor("out", [PADR, FA], fp16, kind="ExternalOutput").ap()

    # device-side views:
    #   xT as [p=128, chunk=4, rows]  (partition p owns d = c*128 + p)
    xTv = xT.rearrange("(c p) r -> p c r", c=4, p=P)
    #   out partition-major: hbm row p*NCHUNK + q <-> graph row q*128 + p
    outv = out.rearrange("(p q) f -> p q f", q=NCHUNK)

    with tile.TileContext(nc) as tc:
        with (
            tc.tile_pool(name="const", bufs=1) as constp,
            tc.tile_pool(name="xin", bufs=3) as xp,
            tc.tile_pool(name="evec", bufs=2) as ep,
            tc.tile_pool(name="pexp", bufs=2) as pp,
            tc.tile_pool(name="osb", bufs=3) as op_,
            tc.tile_pool(name="psS", bufs=2, space="PSUM") as psS,
            tc.tile_pool(name="psH", bufs=2, space="PSUM") as psH,
        ):
            w1_sb = constp.tile([P, 4, C], fp16)
            nc.sync.dma_start(out=w1_sb,
                              in_=w1rep.rearrange("(c p) f -> p c f", c=4))
            t_sb = constp.tile([2 * C, 2], fp32)
            nc.sync.dma_start(out=t_sb, in_=tbias)
            vh_sb = constp.tile([2 * C, 2, FA], fp16)
            nc.sync.dma_start(out=vh_sb,
                              in_=vha.rearrange("v p f -> p v f"))

            # one block per group (<=4 pairs). Pairs stack two-deep on
            # PSUM partitions: pair bb lives at partitions 64*(bb//2).. and
            # bank slot bb%2, so a 4-pair block fills [128, 2, 512] = 2
            # banks and ONE Prelu/Exp covers all 2048 rows. The s-stage of
            # group g+1 is emitted before the softmax/h-stage of group g so
            # the PE never stalls on ACT.
            nG = len(GROUPS)
            gbase = [sum(GROUPS[:g]) for g in range(nG)]
            state = {}

            def emit_load(g):
                # one transfer per group, issued a full group ahead of use
                r0, r1 = gbase[g] * RPP, (gbase[g] + GROUPS[g]) * RPP
                xt = xp.tile([P, 4, GROUPS[g] * RPP], fp16, tag="xt")
                nc.sync.dma_start(out=xt, in_=xTv[:, :, r0:r1])
                state[g] = xt

            def emit_s(g):
                nb = GROUPS[g]
                xt = state[g]
                # sT[p, r] = x[r, :] . w1 (64 partition copies feed Prelu's
                # bias layout); chunk-major so consecutive matmuls reuse
                # the same loaded w1 chunk (same-weight matmuls skip the
                # weight-load cost)
                psum_s = psS.tile([P, 2, RPP], fp32)
                for c in range(4):
                    for bb in range(nb):
                        hb, sb = 64 * (bb // 2), bb % 2
                        nc.tensor.matmul(psum_s[hb:hb + C, sb, :],
                                         w1_sb[:, c, :],
                                         xt[:, c, bb * RPP:(bb + 1) * RPP],
                                         start=(c == 0), stop=(c == 3))
                state[g] = (xt, psum_s)

            def emit_rest(g):
                nb = GROUPS[g]
                _, psum_s = state.pop(g)
                nparts = C if nb <= 2 else P
                nslots = min(nb, 2)
                osb = op_.tile([P, nb * (RPP // P), FA], fp16, tag="osb",
                               name="osb")
                # eT = leaky_relu(sT + t_j): t is a per-partition bias
                eT = ep.tile([P, 2, RPP], fp16, tag="eT")
                nc.scalar.activation(out=eT[:nparts, :nslots, :],
                                     in_=psum_s[:nparts, :nslots, :],
                                     func=Act.Prelu,
                                     bias=t_sb[:nparts, 0:1], scale=1.0,
                                     alpha=ALPHA)
                # pexpT = exp(eT + MSHIFT), shifted into fp16-safe range;
                # the shift cancels in h/z on host
                pexpT = pp.tile([P, 2, RPP], fp16, tag="pexpT")
                nc.scalar.activation(out=pexpT[:nparts, :nslots, :],
                                     in_=eT[:nparts, :nslots, :],
                                     func=Act.Exp, bias=t_sb[:nparts, 1:2],
                                     scale=1.0)
                # h|z chunks of 128 rows: lhsT = pexpT slice (already
                # transposed layout), rhs = [vh | ones]
                for half in range(2 * nb):
                    bb = half // 2
                    hv, sb = bb // 2, bb % 2
                    ps_h = psH.tile([P, 2, RPP], fp32, name="ps_h",
                                    tag="psH")
                    for kk in range(2):
                        q = (half % 2) * 2 + kk
                        # full-height lhsT (both stacked pair-groups); the
                        # rhs variant hv zero-masks the other group's rows
                        nc.tensor.matmul(
                            ps_h[:, kk, :FA],
                            pexpT[:, sb, q * P:(q + 1) * P] if nb > 2
                            else pexpT[:C, sb, q * P:(q + 1) * P],
                            vh_sb[:, hv, :] if nb > 2
                            else vh_sb[:C, 0, :], start=True, stop=True)
                    oq = bb * 4 + (half % 2) * 2
                    nc.vector.tensor_copy(osb[:, oq:oq + 2, :],
                                          ps_h[:, :, :FA])
                q0 = gbase[g] * (RPP // P)
                nc.scalar.dma_start(out=outv[:, q0:q0 + 4 * nb, :], in_=osb)

            emit_load(0)
            emit_load(1)
            emit_s(0)
            for g in range(1, nG + 1):
                if g < nG:
                    if g + 1 < nG:
                        emit_load(g + 1)
                    emit_s(g)
                emit_rest(g - 1)

    nc.compile()
    return nc


def _get_nc():
    if "nc" not in _CACHE:
        _CACHE["nc"] = _build_nc()
    return _CACHE["nc"]


def _prep_inputs(graph_node, virtual_node, W, a):
    f32, f16 = np.float32, np.float16
    W = np.asarray(W, f32)
    a = np.asarray(a, f32)
    a1 = a[:F, 0]
    a2 = a[F:, 0]
    w1 = (W @ a1).astype(f32)                             # (D,)
    vh = (np.asarray(virtual_node, f32) @ W).astype(f32)  # (C, F)
    t = (vh @ a2).astype(f32)                             # (C,)
    w1rep = np.ascontiguousarray(
        np.broadcast_to(w1[:, None].astype(f16), (D, C)))
    t2 = np.concatenate([t, t])
    tbias = np.stack([t2, np.full((2 * C,), MSHIFT, f32)], axis=1)
    tbias = np.ascontiguousarray(tbias, dtype=f32)
    vh1 = np.concatenate([vh, np.ones((C, 1), f32)], axis=1).astype(f16)
    z64 = np.zeros_like(vh1)
    vha = np.ascontiguousarray(np.stack([
        np.concatenate([vh1, z64], axis=0),      # top pair-group live
        np.concatenate([z64, vh1], axis=0),      # bottom pair-group live
    ]))

    X = np.asarray(graph_node, f32).astype(f16)
    in_maps = []
    for c in range(NCORES):
        xT = np.zeros((D, PADR), f16)
        xT[:, :SHARD] = X[c * SHARD:(c + 1) * SHARD].T
        in_maps.append({"xT": xT, "w1rep": w1rep, "tbias": tbias,
                        "vha": vha})
    return in_maps


def _postprocess(res):
    outs = []
    for c in range(NCORES):
        o = res[c]["out"]                                 # (PADR, FA) fp16
        # un-permute partition-major rows: hbm row p*NCHUNK+q -> q*128+p
        o = np.ascontiguousarray(
            o.reshape(P, NCHUNK, FA).transpose(1, 0, 2).reshape(PADR, FA)
        )[:SHARD].astype(np.float32)
        outs.append(o[:, :F] / o[:, F:F + 1])
    return np.concatenate(outs, axis=0)


def _run(inputs, trace=False, **trace_kwargs):
    from concourse.bass_utils import run_bass_kernel_spmd

    nc = _get_nc()
    in_maps = _prep_inputs(**inputs)
    res = run_bass_kernel_spmd(nc, in_maps, list(range(NCORES)),
                               trace=trace, **trace_kwargs)
    out = _postprocess([res.results[c] for c in range(NCORES)])
    return out, res


def kernel(**inputs) -> np.ndarray:
    out, _ = _run(inputs)
    return out


# revision 15
# speedup vs baseline: 1.1484x; 1.0891x over previous
"""Trainium2 Bass kernel for virtual-node GAT attention (gnn_message_passing).

Reference semantics (N=100000, C=64, D=512, F=256):
    gh  = graph_node @ W            # (N, F)
    vh  = virtual_node @ W          # (C, F)
    e   = gh @ a1 + (vh @ a2)^T     # (N, C)
    e   = leaky_relu(e, 0.2)
    att = softmax(e, axis=1)
    out = att @ vh                  # (N, F)

Algebraic identity: gh only enters via gh @ a1 = graph_node @ (W @ a1), so
the (N,D)@(D,F) matmul is never needed. Host precomputes the tiny shared
tables w1 = W@a1 (D,), vh (C,F), t = vh@a2 (C,).

Transposed device pipeline: the host stages x TRANSPOSED (xT [D, rows],
fp16), so every per-row stage runs with rows on the matmul free dim and no
on-chip transpose is ever needed:
  sT   = w1rep^T @ xT          PE: 4 accumulating 128-contraction matmuls
                               per 512 rows, lhsT = w1 chunk replicated 64
                               wide -> sT in PSUM [64 (redundant), rows]
  eT   = Prelu(sT + t)         ACT: one op per 1024 rows; t is a per-
                               partition bias [64,1] in this layout
  pexpT= Exp(eT - 10.5)        ACT: shift keeps exp(e) inside fp16 range
  h|z  = pexpT^T @ [vh | 1]    PE: pexpT slices [64,128] are ALREADY in
                               lhsT layout; ones-column gives z for free
  osb  = copy h|z              DVE: strided PSUM->SBUF fp16 casts
Host divides h by z (softmax denominator) and casts to fp32; the shift
cancels in the division. fp16 end-to-end rel err ~4e-3 (gate is 2e-2).

Output rows are stored PARTITION-MAJOR in HBM (hbm row p*NCHUNK+q holds
graph row q*128+p) so each partition writes one contiguous ~8KB run per
group instead of 100 separate 514B packets; the host un-permutes with one
cheap reshape. Everything streams fp16: 19.7 MB HBM traffic per core
(13.1 in + 6.6 out) vs 39 MB for the fp32 baseline.

Sharding: graph_node rows split evenly across 8 cores (data parallel),
small tables replicated. No cross-device communication.
"""

import numpy as np

N, D, F, C = 100000, 512, 256, 64
NCORES = 8
SHARD = N // NCORES            # 12500 rows per core
P = 128                        # partitions
RPP = 512                      # rows per pair (one psS bank of fp32)
NPAIR = (SHARD + RPP - 1) // RPP   # 25
PADR = NPAIR * RPP             # 12800 rows per core (zero-padded)
NCHUNK = PADR // P             # 100 output chunks of 128 rows
FA = F + 1                     # 257: h columns + z (softmax denom)
# group sizes in pairs: small first group -> compute starts sooner; small
# tail -> short drain after the final load. Even sizes so pairs batch into
# 2-pair blocks (one Prelu/Exp per 1024 rows); the final pair runs alone.
GROUPS = [2, 4, 4, 4, 4, 4, 2, 1]
assert sum(GROUPS) == NPAIR
ALPHA = 0.2
MSHIFT = -10.5                 # exp argument shift (cancels in softmax);
                               # keeps h = pexp@vh under fp16 max (~9e3
                               # worst row) and z above fp16 normal min

_CACHE = {}


def _build_nc():
    import concourse.bacc as bacc
    import concourse.mybir as mybir
    import concourse.tile as tile

    fp32 = mybir.dt.float32
    fp16 = mybir.dt.float16
    Act = mybir.ActivationFunctionType

    nc = bacc.Bacc("TRN2", target_bir_lowering=False, debug=False,
                   num_devices=NCORES)
    xT = nc.dram_tensor("xT", [D, PADR], fp16, kind="ExternalInput").ap()
    w1rep = nc.dram_tensor("w1rep", [D, C], fp16, kind="ExternalInput").ap()
    tbias = nc.dram_tensor("tbias", [2 * C, 2], fp32, kind="ExternalInput").ap()
    vha = nc.dram_tensor("vha", [2, 2 * C, FA], fp16, kind="ExternalInput").ap()
    out = nc.dram_tensor("out", [PADR, FA], fp16, kind="ExternalOutput").ap()

    # device-side views:
    #   xT as [p=128, chunk=4, rows]  (partition p owns d = c*128 + p)
    xTv = xT.rearrange("(c p) r -> p c r", c=4, p=P)
    #   out partition-major: hbm row p*NCHUNK + q <-> graph row q*128 + p
    outv = out.rearrange("(p q) f -> p q f", q=NCHUNK)

    with tile.TileContext(nc) as tc:
        with (
            tc.tile_pool(name="const", bufs=1) as constp,
            tc.tile_pool(name="xin", bufs=3) as xp,
            tc.tile_pool(name="evec", bufs=2) as ep,
            tc.tile_pool(name="pexp", bufs=2) as pp,
            tc.tile_pool(name="osb", bufs=3) as op_,
            tc.tile_pool(name="psS", bufs=2, space="PSUM") as psS,
            tc.tile_pool(name="psH", bufs=2, space="PSUM") as psH,
        ):
            w1_sb = constp.tile([P, 4, C], fp16)
            nc.sync.dma_start(out=w1_sb,
                              in_=w1rep.rearrange("(c p) f -> p c f", c=4))
            t_sb = constp.tile([2 * C, 2], fp32)
            nc.sync.dma_start(out=t_sb, in_=tbias)
            vh_sb = constp.tile([2 * C, 2, FA], fp16)
            nc.sync.dma_start(out=vh_sb,
                              in_=vha.rearrange("v p f -> p v f"))

            # one block per group (<=4 pairs). Pairs stack two-deep on
            # PSUM partitions: pair bb lives at partitions 64*(bb//2).. and
            # bank slot bb%2, so a 4-pair block fills [128, 2, 512] = 2
            # banks and ONE Prelu/Exp covers all 2048 rows. The s-stage of
            # group g+1 is emitted before the softmax/h-stage of group g so
            # the PE never stalls on ACT.
            nG = len(GROUPS)
            gbase = [sum(GROUPS[:g]) for g in range(nG)]
            state = {}

            def emit_load(g):
                # one transfer per group, issued a full group ahead of use
                r0, r1 = gbase[g] * RPP, (gbase[g] + GROUPS[g]) * RPP
                xt = xp.tile([P, 4, GROUPS[g] * RPP], fp16, tag="xt")
                nc.sync.dma_start(out=xt, in_=xTv[:, :, r0:r1])
                state[g] = xt

            def emit_s(g):
                nb = GROUPS[g]
                xt = state[g]
                # sT[p, r] = x[r, :] . w1 (64 partition copies feed Prelu's
                # bias layout); chunk-major so consecutive matmuls reuse
                # the same loaded w1 chunk (same-weight matmuls skip the
                # weight-load cost)
                psum_s = psS.tile([P, 2, RPP], fp32)
                for c in range(4):
                    for bb in range(nb):
                        hb, sb = 64 * (bb // 2), bb % 2
                        nc.tensor.matmul(psum_s[hb:hb + C, sb, :],
                                         w1_sb[:, c, :],
                                         xt[:, c, bb * RPP:(bb + 1) * RPP],
                                         start=(c == 0), stop=(c == 3))
                state[g] = (xt, psum_s)

            def emit_rest(g):
                nb = GROUPS[g]
                _, psum_s = state.pop(g)
                nparts = C if nb <= 2 else P
                nslots = min(nb, 2)
                osb = op_.tile([P, nb * (RPP // P), FA], fp16, tag="osb",
                               name="osb")
                # eT = leaky_relu(sT + t_j): t is a per-partition bias
                eT = ep.tile([P, 2, RPP], fp16, tag="eT")
                nc.scalar.activation(out=eT[:nparts, :nslots, :],
                                     in_=psum_s[:nparts, :nslots, :],
                                     func=Act.Prelu,
                                     bias=t_sb[:nparts, 0:1], scale=1.0,
                                     alpha=ALPHA)
                # pexpT = exp(eT + MSHIFT), shifted into fp16-safe range;
                # the shift cancels in h/z on host
                pexpT = pp.tile([P, 2, RPP], fp16, tag="pexpT")
                nc.scalar.activation(out=pexpT[:nparts, :nslots, :],
                                     in_=eT[:nparts, :nslots, :],
                                     func=Act.Exp, bias=t_sb[:nparts, 1:2],
                                     scale=1.0)
                # h|z chunks of 128 rows: lhsT = pexpT slice (already
                # transposed layout), rhs = [vh | ones]
                if nb > 2:
                    # full-height lhsT (both stacked pair-groups): the top
                    # pair (rhs variant 0) and bottom pair (variant 1) use
                    # IDENTICAL weights, so emitting them back-to-back
                    # makes the second matmul's weight load free
                    for sb in range(2):
                        for t in range(2):
                            psA = psH.tile([P, 2, RPP], fp32, name="psA",
                                           tag="psH")
                            psB = psH.tile([P, 2, RPP], fp32, name="psB",
                                           tag="psH")
                            for kk in range(2):
                                q = 2 * t + kk
                                lhs = pexpT[:, sb, q * P:(q + 1) * P]
                                nc.tensor.matmul(psA[:, kk, :FA], lhs,
                                                 vh_sb[:, 0, :],
                                                 start=True, stop=True)
                                nc.tensor.matmul(psB[:, kk, :FA], lhs,
                                                 vh_sb[:, 1, :],
                                                 start=True, stop=True)
                            nc.vector.tensor_copy(
                                osb[:, sb * 4 + 2 * t:sb * 4 + 2 * t + 2,
                                    :], psA[:, :, :FA])
                            nc.vector.tensor_copy(
                                osb[:, (sb + 2) * 4 + 2 * t:
                                    (sb + 2) * 4 + 2 * t + 2, :],
                                psB[:, :, :FA])
                else:
                    for half in range(2 * nb):
                        bb, t = half // 2, half % 2
                        sb = bb % 2
                        ps_h = psH.tile([P, 2, RPP], fp32, name="ps_h",
                                        tag="psH")
                        for kk in range(2):
                            q = 2 * t + kk
                            nc.tensor.matmul(
                                ps_h[:, kk, :FA],
                                pexpT[:C, sb, q * P:(q + 1) * P],
                                vh_sb[:C, 0, :], start=True, stop=True)
                        nc.vector.tensor_copy(
                            osb[:, bb * 4 + 2 * t:bb * 4 + 2 * t + 2, :],
                            ps_h[:, :, :FA])
                q0 = gbase[g] * (RPP // P)
                nc.scalar.dma_start(out=outv[:, q0:q0 + 4 * nb, :], in_=osb)

            emit_load(0)
            emit_load(1)
            emit_s(0)
            for g in range(1, nG + 1):
                if g < nG:
                    if g + 1 < nG:
                        emit_load(g + 1)
                    emit_s(g)
                emit_rest(g - 1)

    nc.compile()
    return nc


def _get_nc():
    if "nc" not in _CACHE:
        _CACHE["nc"] = _build_nc()
    return _CACHE["nc"]


def _prep_inputs(graph_node, virtual_node, W, a):
    f32, f16 = np.float32, np.float16
    W = np.asarray(W, f32)
    a = np.asarray(a, f32)
    a1 = a[:F, 0]
    a2 = a[F:, 0]
    w1 = (W @ a1).astype(f32)                             # (D,)
    vh = (np.asarray(virtual_node, f32) @ W).astype(f32)  # (C, F)
    t = (vh @ a2).astype(f32)                             # (C,)
    w1rep = np.ascontiguousarray(
        np.broadcast_to(w1[:, None].astype(f16), (D, C)))
    t2 = np.concatenate([t, t])
    tbias = np.stack([t2, np.full((2 * C,), MSHIFT, f32)], axis=1)
    tbias = np.ascontiguousarray(tbias, dtype=f32)
    vh1 = np.concatenate([vh, np.ones((C, 1), f32)], axis=1).astype(f16)
    z64 = np.zeros_like(vh1)
    vha = np.ascontiguousarray(np.stack([
        np.concatenate([vh1, z64], axis=0),      # top pair-group live
        np.concatenate([z64, vh1], axis=0),      # bottom pair-group live
    ]))

    X = np.asarray(graph_node, f32).astype(f16)
    in_maps = []
    for c in range(NCORES):
        xT = np.zeros((D, PADR), f16)
        xT[:, :SHARD] = X[c * SHARD:(c + 1) * SHARD].T
        in_maps.append({"xT": xT, "w1rep": w1rep, "tbias": tbias,
                        "vha": vha})
    return in_maps


def _postprocess(res):
    outs = []
    for c in range(NCORES):
        o = res[c]["out"]                                 # (PADR, FA) fp16
        # un-permute partition-major rows: hbm row p*NCHUNK+q -> q*128+p
        o = np.ascontiguousarray(
            o.reshape(P, NCHUNK, FA).transpose(1, 0, 2).reshape(PADR, FA)
        )[:SHARD].astype(np.float32)
        outs.append(o[:, :F] / o[:, F:F + 1])
    return np.concatenate(outs, axis=0)


def _run(inputs, trace=False, **trace_kwargs):
    from concourse.bass_utils import run_bass_kernel_spmd

    nc = _get_nc()
    in_maps = _prep_inputs(**inputs)
    res = run_bass_kernel_spmd(nc, in_maps, list(range(NCORES)),
                               trace=trace, **trace_kwargs)
    out = _postprocess([res.results[c] for c in range(NCORES)])
    return out, res


def kernel(**inputs) -> np.ndarray:
    out, _ = _run(inputs)
    return out


# revision 16
# speedup vs baseline: 1.2048x; 1.0492x over previous
"""Trainium2 Bass kernel for virtual-node GAT attention (gnn_message_passing).

Reference semantics (N=100000, C=64, D=512, F=256):
    gh  = graph_node @ W            # (N, F)
    vh  = virtual_node @ W          # (C, F)
    e   = gh @ a1 + (vh @ a2)^T     # (N, C)
    e   = leaky_relu(e, 0.2)
    att = softmax(e, axis=1)
    out = att @ vh                  # (N, F)

Algebraic identity: gh only enters via gh @ a1 = graph_node @ (W @ a1), so
the (N,D)@(D,F) matmul is never needed. Host precomputes the tiny shared
tables w1 = W@a1 (D,), vh (C,F), t = vh@a2 (C,).

Transposed device pipeline: the host stages x TRANSPOSED (xT [D, rows],
fp16), so every per-row stage runs with rows on the matmul free dim and no
on-chip transpose is ever needed:
  sT   = w1rep^T @ xT          PE: 4 accumulating 128-contraction matmuls
                               per 512 rows, lhsT = w1 chunk replicated 64
                               wide -> sT in PSUM [64 (redundant), rows]
  eT   = Prelu(sT + t)         ACT: one op per 1024 rows; t is a per-
                               partition bias [64,1] in this layout
  pexpT= Exp(eT - 10.5)        ACT: shift keeps exp(e) inside fp16 range
  h|z  = pexpT^T @ [vh | 1]    PE: pexpT slices [64,128] are ALREADY in
                               lhsT layout; ones-column gives z for free
  osb  = copy h|z              DVE: strided PSUM->SBUF fp16 casts
Host divides h by z (softmax denominator) and casts to fp32; the shift
cancels in the division. fp16 end-to-end rel err ~4e-3 (gate is 2e-2).

Output rows are stored PARTITION-MAJOR in HBM (hbm row p*NCHUNK+q holds
graph row q*128+p) so each partition writes one contiguous ~8KB run per
group instead of 100 separate 514B packets; the host un-permutes with one
cheap reshape. Everything streams fp16: 19.7 MB HBM traffic per core
(13.1 in + 6.6 out) vs 39 MB for the fp32 baseline.

Sharding: graph_node rows split evenly across 8 cores (data parallel),
small tables replicated. No cross-device communication.
"""

import numpy as np

N, D, F, C = 100000, 512, 256, 64
NCORES = 8
SHARD = N // NCORES            # 12500 rows per core
P = 128                        # partitions
RPP = 512                      # rows per pair (one psS bank of fp32)
NPAIR = (SHARD + RPP - 1) // RPP   # 25
PADR = NPAIR * RPP             # 12800 rows per core (zero-padded)
NCHUNK = PADR // P             # 100 output chunks of 128 rows
FA = F + 1                     # 257: h columns + z (softmax denom)
# group sizes in pairs: small first group -> compute starts sooner; small
# tail -> short drain after the final load. Even sizes so pairs batch into
# 2-pair blocks (one Prelu/Exp per 1024 rows); the final pair runs alone.
GROUPS = [4, 4, 4, 4, 4, 4, 1]
assert sum(GROUPS) == NPAIR
ALPHA = 0.2
MSHIFT = -10.5                 # exp argument shift (cancels in softmax);
                               # keeps h = pexp@vh under fp16 max (~9e3
                               # worst row) and z above fp16 normal min

_CACHE = {}


def _build_nc():
    import concourse.bacc as bacc
    import concourse.mybir as mybir
    import concourse.tile as tile

    fp32 = mybir.dt.float32
    fp16 = mybir.dt.float16
    Act = mybir.ActivationFunctionType

    nc = bacc.Bacc("TRN2", target_bir_lowering=False, debug=False,
                   num_devices=NCORES)
    xT = nc.dram_tensor("xT", [D, PADR], fp16, kind="ExternalInput").ap()
    w1rep = nc.dram_tensor("w1rep", [D, C], fp16, kind="ExternalInput").ap()
    tbias = nc.dram_tensor("tbias", [2 * C, 2], fp32, kind="ExternalInput").ap()
    vha = nc.dram_tensor("vha", [2, 2 * C, FA], fp16, kind="ExternalInput").ap()
    out = nc.dram_tensor("out", [PADR, FA], fp16, kind="ExternalOutput").ap()

    # device-side views:
    #   xT as [p=128, chunk=4, rows]  (partition p owns d = c*128 + p)
    xTv = xT.rearrange("(c p) r -> p c r", c=4, p=P)
    #   out partition-major: hbm row p*NCHUNK + q <-> graph row q*128 + p
    outv = out.rearrange("(p q) f -> p q f", q=NCHUNK)

    with tile.TileContext(nc) as tc:
        with (
            tc.tile_pool(name="const", bufs=1) as constp,
            tc.tile_pool(name="xin", bufs=3) as xp,
            tc.tile_pool(name="evec", bufs=2) as ep,
            tc.tile_pool(name="pexp", bufs=2) as pp,
            tc.tile_pool(name="osb", bufs=3) as op_,
            tc.tile_pool(name="psS", bufs=2, space="PSUM") as psS,
            tc.tile_pool(name="psH", bufs=2, space="PSUM") as psH,
        ):
            w1_sb = constp.tile([P, 4, C], fp16)
            nc.sync.dma_start(out=w1_sb,
                              in_=w1rep.rearrange("(c p) f -> p c f", c=4))
            t_sb = constp.tile([2 * C, 2], fp32)
            nc.sync.dma_start(out=t_sb, in_=tbias)
            vh_sb = constp.tile([2 * C, 2, FA], fp16)
            nc.sync.dma_start(out=vh_sb,
                              in_=vha.rearrange("v p f -> p v f"))

            # one block per group (<=4 pairs). Pairs stack two-deep on
            # PSUM partitions: pair bb lives at partitions 64*(bb//2).. and
            # bank slot bb%2, so a 4-pair block fills [128, 2, 512] = 2
            # banks and ONE Prelu/Exp covers all 2048 rows. The s-stage of
            # group g+1 is emitted before the softmax/h-stage of group g so
            # the PE never stalls on ACT.
            nG = len(GROUPS)
            gbase = [sum(GROUPS[:g]) for g in range(nG)]
            state = {}

            def emit_load(g):
                # one transfer per group, issued a full group ahead of use
                r0, r1 = gbase[g] * RPP, (gbase[g] + GROUPS[g]) * RPP
                xt = xp.tile([P, 4, GROUPS[g] * RPP], fp16, tag="xt")
                nc.sync.dma_start(out=xt, in_=xTv[:, :, r0:r1])
                state[g] = xt

            def emit_s(g):
                nb = GROUPS[g]
                xt = state[g]
                # sT[p, r] = x[r, :] . w1 (64 partition copies feed Prelu's
                # bias layout); chunk-major so consecutive matmuls reuse
                # the same loaded w1 chunk (same-weight matmuls skip the
                # weight-load cost)
                psum_s = psS.tile([P, 2, RPP], fp32)
                for c in range(4):
                    for bb in range(nb):
                        hb, sb = 64 * (bb // 2), bb % 2
                        nc.tensor.matmul(psum_s[hb:hb + C, sb, :],
                                         w1_sb[:, c, :],
                                         xt[:, c, bb * RPP:(bb + 1) * RPP],
                                         start=(c == 0), stop=(c == 3))
                state[g] = (xt, psum_s)

            def emit_rest(g):
                nb = GROUPS[g]
                _, psum_s = state.pop(g)
                nparts = C if nb <= 2 else P
                nslots = min(nb, 2)
                osb = op_.tile([P, nb * (RPP // P), FA], fp16, tag="osb",
                               name="osb")
                # eT = leaky_relu(sT + t_j): t is a per-partition bias
                eT = ep.tile([P, 2, RPP], fp16, tag="eT")
                nc.scalar.activation(out=eT[:nparts, :nslots, :],
                                     in_=psum_s[:nparts, :nslots, :],
                                     func=Act.Prelu,
                                     bias=t_sb[:nparts, 0:1], scale=1.0,
                                     alpha=ALPHA)
                # pexpT = exp(eT + MSHIFT), shifted into fp16-safe range;
                # the shift cancels in h/z on host
                pexpT = pp.tile([P, 2, RPP], fp16, tag="pexpT")
                nc.scalar.activation(out=pexpT[:nparts, :nslots, :],
                                     in_=eT[:nparts, :nslots, :],
                                     func=Act.Exp, bias=t_sb[:nparts, 1:2],
                                     scale=1.0)
                # h|z chunks of 128 rows: lhsT = pexpT slice (already
                # transposed layout), rhs = [vh | ones]
                if nb > 2:
                    # full-height lhsT (both stacked pair-groups): the top
                    # pair (rhs variant 0) and bottom pair (variant 1) use
                    # IDENTICAL weights, so emitting them back-to-back
                    # makes the second matmul's weight load free
                    for sb in range(2):
                        for t in range(2):
                            psA = psH.tile([P, 2, RPP], fp32, name="psA",
                                           tag="psH")
                            psB = psH.tile([P, 2, RPP], fp32, name="psB",
                                           tag="psH")
                            for kk in range(2):
                                q = 2 * t + kk
                                lhs = pexpT[:, sb, q * P:(q + 1) * P]
                                nc.tensor.matmul(psA[:, kk, :FA], lhs,
                                                 vh_sb[:, 0, :],
                                                 start=True, stop=True)
                                nc.tensor.matmul(psB[:, kk, :FA], lhs,
                                                 vh_sb[:, 1, :],
                                                 start=True, stop=True)
                            nc.vector.tensor_copy(
                                osb[:, sb * 4 + 2 * t:sb * 4 + 2 * t + 2,
                                    :], psA[:, :, :FA])
                            nc.vector.tensor_copy(
                                osb[:, (sb + 2) * 4 + 2 * t:
                                    (sb + 2) * 4 + 2 * t + 2, :],
                                psB[:, :, :FA])
                else:
                    for half in range(2 * nb):
                        bb, t = half // 2, half % 2
                        sb = bb % 2
                        ps_h = psH.tile([P, 2, RPP], fp32, name="ps_h",
                                        tag="psH")
                        for kk in range(2):
                            q = 2 * t + kk
                            nc.tensor.matmul(
                                ps_h[:, kk, :FA],
                                pexpT[:C, sb, q * P:(q + 1) * P],
                                vh_sb[:C, 0, :], start=True, stop=True)
                        nc.vector.tensor_copy(
                            osb[:, bb * 4 + 2 * t:bb * 4 + 2 * t + 2, :],
                            ps_h[:, :, :FA])
                q0 = gbase[g] * (RPP // P)
                nc.scalar.dma_start(out=outv[:, q0:q0 + 4 * nb, :], in_=osb)

            emit_load(0)
            emit_load(1)
            emit_s(0)
            for g in range(1, nG + 1):
                if g < nG:
                    if g + 1 < nG:
                        emit_load(g + 1)
                    emit_s(g)
                emit_rest(g - 1)

    nc.compile()
    return nc


def _get_nc():
    if "nc" not in _CACHE:
        _CACHE["nc"] = _build_nc()
    return _CACHE["nc"]


def _prep_inputs(graph_node, virtual_node, W, a):
    f32, f16 = np.float32, np.float16
    W = np.asarray(W, f32)
    a = np.asarray(a, f32)
    a1 = a[:F, 0]
    a2 = a[F:, 0]
    w1 = (W @ a1).astype(f32)                             # (D,)
    vh = (np.asarray(virtual_node, f32) @ W).astype(f32)  # (C, F)
    t = (vh @ a2).astype(f32)                             # (C,)
    w1rep = np.ascontiguousarray(
        np.broadcast_to(w1[:, None].astype(f16), (D, C)))
    t2 = np.concatenate([t, t])
    tbias = np.stack([t2, np.full((2 * C,), MSHIFT, f32)], axis=1)
    tbias = np.ascontiguousarray(tbias, dtype=f32)
    vh1 = np.concatenate([vh, np.ones((C, 1), f32)], axis=1).astype(f16)
    z64 = np.zeros_like(vh1)
    vha = np.ascontiguousarray(np.stack([
        np.concatenate([vh1, z64], axis=0),      # top pair-group live
        np.concatenate([z64, vh1], axis=0),      # bottom pair-group live
    ]))

    X = np.asarray(graph_node, f32).astype(f16)
    in_maps = []
    for c in range(NCORES):
        xT = np.zeros((D, PADR), f16)
        xT[:, :SHARD] = X[c * SHARD:(c + 1) * SHARD].T
        in_maps.append({"xT": xT, "w1rep": w1rep, "tbias": tbias,
                        "vha": vha})
    return in_maps


def _postprocess(res):
    outs = []
    for c in range(NCORES):
        o = res[c]["out"]                                 # (PADR, FA) fp16
        # un-permute partition-major rows: hbm row p*NCHUNK+q -> q*128+p
        o = np.ascontiguousarray(
            o.reshape(P, NCHUNK, FA).transpose(1, 0, 2).reshape(PADR, FA)
        )[:SHARD].astype(np.float32)
        outs.append(o[:, :F] / o[:, F:F + 1])
    return np.concatenate(outs, axis=0)


def _run(inputs, trace=False, **trace_kwargs):
    from concourse.bass_utils import run_bass_kernel_spmd

    nc = _get_nc()
    in_maps = _prep_inputs(**inputs)
    res = run_bass_kernel_spmd(nc, in_maps, list(range(NCORES)),
                               trace=trace, **trace_kwargs)
    out = _postprocess([res.results[c] for c in range(NCORES)])
    return out, res


def kernel(**inputs) -> np.ndarray:
    out, _ = _run(inputs)
    return out
